# revision 1
# baseline (speedup 1.0000x reference)
"""Trainium2 Bass kernel for the DeepHit-style survival loss.

Math (derived from the reference):
  For each sample i with duration d, event e (u = e>0, st = clip(e-1,0,3)):
    r[k]   = 1 - s[k],  s[k] = sum_c phi[i,c,k]
    lse[k] = log(sum_c e^{phi[i,c,k]} + e^{r[k]})
    loss_i = sum_{k<=d} lse[k] + sum_{k<=d-u} s[k] - u*phi[i,st,d] + (u - d - 1)
  output = mean_i loss_i

Device mapping (per core, 8192 samples = 64 tiles of 128 samples on
partitions; per-octet 2MiB DMAs, software-pipelined; octets 0 and 7
split their DMA/exp in half to shorten pipeline fill and drain):
  - NO dtype cast anywhere: ACT reads f32 directly (its cost is
    dtype-independent), and the s-matmul reads the stride-2 uint16 view
    of phi's high half-words (= free bf16 truncation; unbiased on
    zero-mean phi, tolerance is 2e-2)
  - PE: s = sum_c phi_c via bf16 identity-matmuls into a per-octet PSUM
    tile; se = sum_c e^phi + e^(1-s) accumulated into a second PSUM
    tile (separate tiles: dependencies are tracked per whole tile, so
    sharing one tile would false-serialize er's read vs emm's write)
  - ACT (per octet): exp over phi (FD=4096, f32 in / f16 out),
    er = e^(1-s) (FD=1024, bf16 out, fused affine scale=-1 bias=1),
    lse = ln(se) IN-PLACE (FD=1024)
  - DVE: per tile two 128-col scalar_tensor_tensor masked sums with
    accum_out (iota <= d-u over s, iota <= d over lse)
  - gather u*phi[st,d]: one GPSIMD ap_gather per chunk with
    host-prepared chunk-relative int16 indices; each 16-partition group
    applies its 16 stored indices to all its partitions, so the host
    extracts the [p, ti*16 + p%16] diagonal and applies the u mask.
    ap_gather's ISA-lowered operands are invisible to the Tile
    dependency tracker, so a tracked Pool read fence (pinned with a
    no-sync edge) fronts each gather and a tracked Pool write into a
    spare gout column gates the readback DMA
  - PE p-state warmup: dummy matmuls on memset weights keep the PE ramp
    hot through the first real matmul dispatch
  - host: sums partials in f64, adds sum(u - d - 1), divides by N

Sharding: pure data parallel over N across 8 cores; the final mean is
reduced on the host from per-sample partials.
"""

import os
import sys
import numpy as np

for _p in ("/opt/trn_rl_repo",):
    if _p not in sys.path:
        sys.path.insert(0, _p)

import concourse.bass as bass
import concourse.bacc as bacc
import concourse.tile as tile
from concourse import mybir
from concourse.bass_utils import run_bass_kernel_spmd
from concourse.instruction_name_ordered_set import InstructionNameOrderedSet

N_CORES = 8
N, QCAUSE, K = 65536, 4, 128
S = N // N_CORES          # samples per core = 8192
T = S // 128              # tiles (128 samples each) per core = 64
NOCT = T // 8             # 8 octets of 8 tiles
ROW = QCAUSE * K          # 512 floats per sample

F32 = mybir.dt.float32
F16 = mybir.dt.float16
BF16 = mybir.dt.bfloat16

# PE p-state warmup: dummy identity matmuls keep the PE busy from t~0.3us
# until the first real matmul so the cost model's ramp (full clock only
# after 3us of continuous execution) is hot when real work dispatches.
N_PE_WARM = 42

# Engines whose program order is pinned to emission order via no-sync
# dependency edges (empirically the Tile scheduler does better on its
# own, so this is off).
CHAIN_KEYS = ()

_BUILT = None


def _build_program():
    from contextlib import ExitStack
    import ml_dtypes

    nc = bacc.Bacc(
        "TRN2",
        target_bir_lowering=False,
        debug=False,
    )

    phi_d = nc.dram_tensor("phi", [S, ROW], F32, kind="ExternalInput").ap()
    # host-side per-partition tables packed [dsu | dd]:
    #   dsu = d - u, dd = d (masked-sum thresholds)
    # cp32 packs [dsu | dd | jidx-as-f32-bits] into one input DMA
    cp32_d = nc.dram_tensor("cp32", [128, 2 * T + T // 2], F32, kind="ExternalInput").ap()
    outM_d = nc.dram_tensor("accM", [128, T], F32, kind="ExternalOutput").ap()
    outL_d = nc.dram_tensor("accL", [128, T], F32, kind="ExternalOutput").ap()
    outG_d = nc.dram_tensor("gout", [128, 16 * T + 16], F32, kind="ExternalOutput").ap()

    # Constants baked into the NEFF, packed into one u16 tensor -> one
    # DMA: iota_row (f16, doubles as anything needing 0..511), the
    # interleaved iota_eo pair [2k+1 | 2k] (f16), and the bf16 identity.
    # iota_eo vs threshold D = 2d+1-u: first half (applied to s) = 2k+1
    # -> mask k <= d-u; second half (applied to lse) = 2k -> mask k <= d.
    iota_row = np.tile(np.arange(K, dtype=np.float16), (128, 1))        # [128,128]
    ident_h = np.eye(128, dtype=np.float16)
    ident_b = np.eye(128).astype(ml_dtypes.bfloat16)
    cpack16 = np.concatenate(
        [iota_row.view(np.uint16), ident_b.view(np.uint16)], axis=1
    )                                                                   # [128,640]
    idh_d = nc.inline_tensor(ident_h, name="idh").ap()
    cp16_d = nc.inline_tensor(cpack16, name="cp16").ap()

    is_le = mybir.AluOpType.is_le
    mult = mybir.AluOpType.mult
    Exp = mybir.ActivationFunctionType.Exp
    Log = mybir.ActivationFunctionType.Ln

    # Octets 0-3 and NOCT-1 are processed as two 4-tile chunks (own
    # phi/exp tiles): splitting the exp is ACT-cost-neutral and pulls the
    # front of the pipeline several us earlier, shortening the ACT
    # backlog that otherwise sets the drain; the last octet's split
    # shortens the tail chain. er/ln/psum stay per-octet.
    def chunks_of(o):
        return [(0, 4), (4, 4)] if o in (0, 1, 2, 3, NOCT - 1) else [(0, 8)]

    _chain_last = {}

    def chain(key, binst):
        if key not in CHAIN_KEYS:
            return binst
        prev = _chain_last.get(key)
        if prev is not None:
            s = InstructionNameOrderedSet()
            s.add(prev.ins.name)
            binst.ins.add_nosync_dependencies_from(s)
        _chain_last[key] = binst
        return binst

    with tile.TileContext(nc) as tc, ExitStack() as ctx:
        singles = ctx.enter_context(tc.tile_pool(name="singles", bufs=1))
        phip8 = ctx.enter_context(tc.tile_pool(name="phip8", bufs=4))
        phip4 = ctx.enter_context(tc.tile_pool(name="phip4", bufs=8))
        octp = ctx.enter_context(tc.tile_pool(name="octp", bufs=2))
        erp = ctx.enter_context(tc.tile_pool(name="erp", bufs=4))
        junkp = ctx.enter_context(tc.tile_pool(name="junkp", bufs=8))
        psp_s = ctx.enter_context(tc.tile_pool(name="psS", bufs=2, space="PSUM"))
        psp_e = ctx.enter_context(tc.tile_pool(name="psE", bufs=2, space="PSUM"))

        phiC = {}
        expC = {}
        erB = {}
        psS = {}
        psE = {}

        def dma(o, lo, nt):
            pool = phip8 if nt == 8 else phip4
            t = pool.tile([128, nt, ROW], F32, tag=f"phi{nt}")
            src = phi_d[o * 1024 + lo * 128 : o * 1024 + (lo + nt) * 128, :].rearrange(
                "(t p) r -> p t r", t=nt
            )
            chain("SP", nc.sync.dma_start(out=t, in_=src))
            phiC[(o, lo)] = t

        def dma_all(o):
            for lo, nt in chunks_of(o):
                dma(o, lo, nt)

        def exp_(o, lo, nt):
            e = octp.tile([128, nt * ROW], F16, tag=f"exp{nt}")
            chain(
                "ACT",
                nc.scalar.activation(
                    e, phiC[(o, lo)].rearrange("p t r -> p (t r)"), Exp
                ),
            )
            expC[(o, lo)] = e

        def exp_all(o):
            for lo, nt in chunks_of(o):
                exp_(o, lo, nt)

        def smm(o, lo, nt):
            # s = sum_c phi_c: identity-matmuls over the stride-2 uint16
            # view of phi's high half-words — i.e. free bf16 truncation
            # (unbiased on zero-mean phi; tolerance is 2e-2), since the
            # PE's native-f32 path is 4x slower and f32r needs a rounding
            # producer the pipeline can't afford
            if lo == 0:
                ps = psp_s.tile([128, 1024], F32, tag="ps", name=f"psS{o}")
                psS[o] = ps
            ps = psS[o]
            hi = phiC[(o, lo)].bitcast(mybir.dt.uint16)[:, :, 1::2].bitcast(BF16)
            for g0 in range(0, nt, 4):
                gn = min(4, nt - g0)
                for c in range(4):
                    rhs = hi[:, g0 : g0 + gn, c * K : (c + 1) * K]
                    chain(
                        "PE",
                        nc.tensor.matmul(
                            ps[:, (lo + g0) * K : (lo + g0 + gn) * K],
                            idb,
                            rhs,
                            start=(c == 0),
                            stop=(c == 3),
                        ),
                    )

        def smm_all(o):
            for lo, nt in chunks_of(o):
                smm(o, lo, nt)

        def er_(o):
            e = erp.tile([128, 1024], BF16, tag="er")
            chain(
                "ACT",
                nc.scalar.activation(e, psS[o], Exp, bias=1.0, scale=-1.0),
            )
            erB[o] = e

        def emm_exp(o, lo, nt):
            # se partial = sum_c e^phi via PE accumulation. Separate PSUM
            # tile from s so PE need not wait for er's read (dependencies
            # are tracked per whole tile).
            if lo == 0:
                ps = psp_e.tile([128, 1024], F32, tag="pe", name=f"psE{o}")
                psE[(o, 0)] = ps
            base = lo
            ps = psE[(o, 0)]
            eo = expC[(o, lo)].rearrange("p (t r) -> p t r", t=nt)
            for g0 in range(0, nt, 4):
                gn = min(4, nt - g0)
                for c in range(4):
                    chain(
                        "PE",
                        nc.tensor.matmul(
                            ps[:, (base + g0) * K : (base + g0 + gn) * K],
                            idh,
                            eo[:, g0 : g0 + gn, c * K : (c + 1) * K],
                            start=(c == 0),
                            stop=False,
                        ),
                    )

        def emm_all(o):
            for lo, nt in chunks_of(o):
                emm_exp(o, lo, nt)

        def er_add(o):
            for h_ in range(2):
                chain(
                    "PE",
                    nc.tensor.matmul(
                        psE[(o, 0)][:, h_ * 512 : (h_ + 1) * 512],
                        idb,
                        erB[o][:, h_ * 512 : (h_ + 1) * 512],
                        start=False,
                        stop=True,
                    ),
                )

        def ln_(o):
            ps = psE[(o, 0)]
            chain("ACT", nc.scalar.activation(ps, ps, Log))

        def j12s(o):
            # sum_{k<=d-u} s[k]: mask d-u can be -1 (no terms match)
            for ti in range(8):
                t = o * 8 + ti
                jk = junkp.tile([128, K], F32, tag="j12s")
                chain(
                    "DVE",
                    nc.vector.scalar_tensor_tensor(
                        out=jk,
                        in0=ior,
                        scalar=dsu[:, t : t + 1],
                        in1=psS[o][:, ti * K : (ti + 1) * K],
                        op0=is_le,
                        op1=mult,
                        accum_out=accM[:, t : t + 1],
                    ),
                )

        def j12l(o):
            # sum_{k<=d} lse[k]
            for ti in range(8):
                t = o * 8 + ti
                src_lse = psE[(o, 0)][:, ti * K : (ti + 1) * K]
                jk = junkp.tile([128, K], F32, tag="j12l")
                chain(
                    "DVE",
                    nc.vector.scalar_tensor_tensor(
                        out=jk,
                        in0=ior,
                        scalar=dd[:, t : t + 1],
                        in1=src_lse,
                        op0=is_le,
                        op1=mult,
                        accum_out=accL[:, t : t + 1],
                    ),
                )

        def _after(a, b):
            # pin scheduler order a -> b (same engine); needed because
            # ap_gather's ISA-lowered operands are invisible to the Tile
            # dependency tracker
            s = InstructionNameOrderedSet()
            s.add(a.ins.name)
            b.ins.add_nosync_dependencies_from(s)
            return b

        def j3(o):
            # u*phi[st,d] gather: one GPSIMD ap_gather per chunk. Each
            # 16-partition group applies its 16 stored indices to all its
            # partitions, so the host extracts the [p, ti*16 + p%16]
            # diagonal of the 16x-blown-up output and applies u there.
            # A tracked read fence in front of each gather (Pool executes
            # in order) carries the phi-DMA semaphore the gather itself
            # cannot.
            for ci, (lo, nt) in enumerate(chunks_of(o)):
                fence = nc.gpsimd.tensor_copy(
                    gfence[:, 2 * ci : 2 * ci + 1], phiC[(o, lo)][:, 0, 0:1]
                )
                gslice = gout[:, o * 128 + lo * 16 : o * 128 + (lo + nt) * 16]
                gi = nc.gpsimd.ap_gather(
                    gslice.rearrange("p (i d) -> p i d", d=1),
                    phiC[(o, lo)].rearrange("p t r -> p (t r)").rearrange("p (e d) -> p e d", d=1),
                    jidx[:, o * 8 + lo : o * 8 + lo + nt],
                    channels=128,
                    num_elems=nt * ROW,
                    d=1,
                    num_idxs=nt * 16,
                )
                _after(fence, gi)
                gathers.append(gi)

        # --- prologue ---
        # PE warmup weights come from a memset (no DMA) so the warmup can
        # start at t~0 while the first phi DMA owns the DMA engines.
        wdm = singles.tile([128, 128], F16)
        chain("DVE", nc.vector.memset(wdm, 1.0))

        dma(0, 0, 4)

        idh = singles.tile([128, 128], F16)
        chain("SP", nc.sync.dma_start(out=idh, in_=idh_d))
        cp32 = singles.tile([128, 2 * T + T // 2], F32)
        chain("SP", nc.sync.dma_start(out=cp32, in_=cp32_d))
        jidx = cp32[:, 2 * T :].bitcast(mybir.dt.int16)

        dma(0, 4, 4)
        dma_all(1)

        # remaining constants, packed into two DMAs
        cp16 = singles.tile([128, 2 * K], mybir.dt.uint16)
        chain("SP", nc.sync.dma_start(out=cp16, in_=cp16_d))
        ior = cp16[:, :K].bitcast(F16)
        idb = cp16[:, K:].bitcast(BF16)
        dsu = cp32[:, 0:T]
        dd = cp32[:, T : 2 * T]

        accM = singles.tile([128, T], F32)
        accL = singles.tile([128, T], F32)
        gout = singles.tile([128, 16 * T + 16], F32)
        gfence = singles.tile([128, 16], F32)
        gathers = []

        # One-time engine reads of the constants: the STT encoding has a
        # tiny sync-wait budget and Tile's wait minimization is per-engine,
        # so the DVE/Pool clocks must observe the constant-load DMA sems
        # before their first scalar_tensor_tensor.
        warm = singles.tile([128, K], F16)
        chain("DVE", nc.vector.tensor_copy(warm, ior))
        warm2 = singles.tile([128, 2], F32)
        chain("DVE", nc.vector.tensor_copy(warm2[:, 0:1], dsu[:, 0:1]))
        chain("DVE", nc.vector.tensor_copy(warm2[:, 1:2], dd[:, 0:1]))
        warm4 = singles.tile([128, 2], mybir.dt.int16)
        chain("POOL", nc.gpsimd.tensor_copy(warm4, jidx[:, 0:2]))

        # PE p-state warmup
        psd = psp_s.tile([128, 1024], F32, tag="ps")
        for _ in range(N_PE_WARM):
            chain(
                "PE", nc.tensor.matmul(psd[:, 0:128], wdm, wdm, start=True, stop=True)
            )

        dma_all(2)
        dma_all(3)
        for lo, nt in chunks_of(0):
            exp_(0, lo, nt)
            smm(0, lo, nt)
        er_(0)

        # --- software-pipelined steady state ---
        for o in range(NOCT):
            if o + 4 < NOCT:
                dma_all(o + 4)
            nxt = chunks_of(o + 1) if o + 1 < NOCT else []
            if o == 0:
                # ln(0) fills the ACT idle while dma(1) is in flight
                emm_all(0)
                er_add(0)
                ln_(0)
                exp_all(1)
                smm_all(1)
                er_(1)
                j12s(0)
                j3(0)
                continue
            if len(nxt) == 2:
                # split next octet: each half gets its own phi/exp tiles
                # so the tail chain is half-sized
                exp_(o + 1, *nxt[0])
                j12l(o - 1)
                emm_all(o)
                er_add(o)
                ln_(o)
                exp_(o + 1, *nxt[1])
                smm_all(o + 1)
                er_(o + 1)
            elif nxt:
                exp_all(o + 1)
                j12l(o - 1)
                emm_all(o)
                er_add(o)
                smm_all(o + 1)
                ln_(o)
                er_(o + 1)
            else:
                j12l(o - 1)
                emm_all(o)
                er_add(o)
                ln_(o)
            j12s(o)
            j3(o)
            if o == 3:
                # first half of the gather results is complete; ship it
                # now so only half the readback sits on the tail
                with tc.high_priority():
                    marker_a = nc.gpsimd.tensor_copy(
                        gout[:, 16 * T + 2 : 16 * T + 3], gfence[:, 0:1]
                    )
                for gi in gathers:
                    _after(gi, marker_a)
                chain("SP", nc.sync.dma_start(out=outG_d[:, 0:512], in_=gout[:, 0:512]))
            if o == NOCT - 1:
                # gate the gout readback behind a tracked Pool write into
                # the spare column, ordered after every gather
                with tc.high_priority():
                    marker = nc.gpsimd.tensor_copy(
                        gout[:, 16 * T : 16 * T + 1], gfence[:, 0:1]
                    )
                for gi in gathers:
                    _after(gi, marker)
                chain("SP", nc.sync.dma_start(out=outM_d, in_=accM))
                chain(
                    "SP",
                    nc.sync.dma_start(
                        out=outG_d[:, 512 : 16 * T + 16],
                        in_=gout[:, 512 : 16 * T + 16],
                    ),
                )
        j12l(NOCT - 1)

        chain("SP", nc.sync.dma_start(out=outL_d, in_=accL))

    # Both Exp and Ln live in the "natural_log_exp_and_others" ACT table
    # set, but the table-load pass picks a set per function greedily and
    # would thrash 2 LoadActFuncSet (~1.3us each) per octet. Restrict the
    # registry (preserving set indices!) so both resolve to the combined
    # set -> a single hoisted load.
    import concourse.bacc as _bacc_mod

    real_get = _bacc_mod.get_activation_tables

    def _only_combined(arch):
        tabs = real_get(arch)
        return {
            name: (fns if name == "natural_log_exp_and_others" else set())
            for name, fns in tabs.items()
        }

    _bacc_mod.get_activation_tables = _only_combined
    try:
        nc.finalize()
    finally:
        _bacc_mod.get_activation_tables = real_get
    return nc


def _get_program():
    global _BUILT
    if _BUILT is None:
        _BUILT = _build_program()
    return _BUILT


def kernel(phi, idx_durations, events):
    phi = np.ascontiguousarray(np.asarray(phi), dtype=np.float32)
    d = np.asarray(idx_durations).astype(np.int64)
    e = np.asarray(events).astype(np.int64)
    u = (e > 0).astype(np.int64)
    st = np.clip(e - 1, 0, QCAUSE - 1)

    nc = _get_program()

    in_maps = []
    for c in range(N_CORES):
        sl = slice(c * S, (c + 1) * S)
        dc, uc, stc = d[sl], u[sl], st[sl]
        dsu = (dc - uc).reshape(T, 128).T.astype(np.float32)
        dd = dc.reshape(T, 128).T.astype(np.float32)
        cp32 = np.concatenate([dsu, dd], axis=1)
        # chunk-relative gather index: ti within its DMA chunk (octets 0
        # and 7 are split in half), then *ROW, plus st*128+d in the row
        ti = np.arange(T) % 8
        ti_rel = ti.copy()
        for o in (0, NOCT - 1):
            mask = (np.arange(T) // 8) == o
            ti_rel[mask] = ti[mask] % 4
        jix = stc * K + dc + ti_rel[(np.arange(S) // 128)] * ROW
        jidx = np.where(uc > 0, jix, 0).reshape(T, 128).T.astype(np.int16)
        cp32 = np.concatenate(
            [cp32, np.ascontiguousarray(jidx).view(np.float32)], axis=1
        )
        in_maps.append(
            {
                "phi": phi[sl].reshape(S, ROW),
                "cp32": np.ascontiguousarray(cp32),
            }
        )

    trace = os.environ.get("BASS_PROFILE") == "1"
    kw = {}
    if trace:
        tmpdir = os.environ.get("BASS_TRACE_DIR") or None
        kw = dict(trace=True, tmpdir=tmpdir)
    res = run_bass_kernel_spmd(nc, in_maps, list(range(N_CORES)), **kw)
    if trace and res.exec_time_ns is not None:
        print(f"HW exec time: {res.exec_time_ns} ns", file=sys.stderr)

    pmod = np.arange(128) % 16
    total = 0.0
    for c in range(N_CORES):
        r = res.results[c]
        total += (
            np.asarray(r["accM"], dtype=np.float64).sum()
            + np.asarray(r["accL"], dtype=np.float64).sum()
        )
        # gout[p, t*16 + p%16] = phi[sample(p,t), st, d]; apply u and sum
        g = np.asarray(r["gout"], dtype=np.float64)[:, : 16 * T].reshape(128, T, 16)
        gd = np.take_along_axis(g, pmod[:, None, None], axis=2)[:, :, 0]  # [128, T]
        uc = u[c * S : (c + 1) * S].reshape(T, 128).T
        total -= float((gd * uc).sum())
    total += float((u - d - 1).sum())
    return np.float32(total / N)


if __name__ == "__main__":
    rng = np.random.default_rng(0)
    phi = rng.standard_normal((N, QCAUSE, K), dtype=np.float32)
    d = rng.integers(0, K, size=(N,)).astype(np.int64)
    e = rng.integers(0, QCAUSE + 1, size=(N,)).astype(np.int64)
    print(kernel(phi, d, e))

